# revision 63
# baseline (speedup 1.0000x reference)
"""Multi-head attention (B=4, S=2048, D=1024, H=16) on 8 TRN2 NeuronCores.

Sharding: data-parallel over batch (4) x tensor-parallel over head halves (2).
Core c handles batch b = c//2 and heads [8g, 8g+8) where g = c%2.
Each core computes a partial [S, D] output-projection contribution; the host
sums the two head-group partials per batch.

All activations are passed to the device pre-transposed (feature dim on
partitions) so the kernel needs no on-device transposes:
  - projections contract over d (model dim) with host-transposed x^T,
  - scores are built transposed [k, q] so exp() output feeds the P @ V
    matmul directly as the moving operand,
  - P @ [V | 1] yields the softmax denominator as row 64 of U^T for free,
  - normalized U^T tiles are exactly the stationary layout w_o needs.

Schedule (v2): the scalar engine's exp stream (256 x [128,1024] ACTIVATEs at
~1.34us each = ~342us) is the hard floor; everything else is arranged to keep
both ACT and PE dense:
  - q-chunks 0 and 1 are interleaved into one 32-turn super-round (c-outer,
    then qc, then hp) so the K/V/Q projection pieces spread at <=2 per turn
    instead of all landing in q-chunk 0.
  - P@V for a turn is emitted one turn later (pending queue), so its exp
    dependency is already satisfied and never head-of-line-blocks the
    in-order PE stream; this also keeps the PE queue backed up so the
    row-tiled score matmul pairs co-start (co-execution needs both
    LDWEIGHTS done before the first MM issues).
  - projection/wo pieces are emitted *inside* turns (between the two
    kp-blocks and after them), never in a burst.
  - wo(qc0) runs during qc2's turns, wo(qc1)+wo(qc2) during qc3's; only
    wo(qc3) (8 pieces) drains in the tail.

Matmul operands are bf16 (fp32 PSUM accumulation). Head pairs share the PE
array via row tiling (partitions 0-63 / 64-127) so the DK=64 score matmuls
run concurrently when co-issued.

PSUM budget (8 banks): 3 x [128,1024] score tiles (6 banks) + one shared
2-slot pool (2 banks) for every [<=128,512] accumulator; attention U
accumulates in SBUF via DVE adds of 4-k-tile PSUM partials so no PSUM slot
is held for long.
"""

import numpy as np

B, S, D, H = 4, 2048, 1024, 16
DK = D // H          # 64
G = 2                # head groups (tensor-parallel degree per batch)
HL = H // G          # 8 local heads per core
DV = HL * DK         # 512 local value dim
N_CORES = 8

_cached = {}


def _build():
    import concourse.bass as bass
    import concourse.tile as tile
    from concourse import bacc, mybir

    f32 = mybir.dt.float32
    bf16 = mybir.dt.bfloat16
    EXP = mybir.ActivationFunctionType.Exp

    nc = bacc.Bacc("TRN2", target_bir_lowering=False, debug=False,
                   num_devices=N_CORES)

    xqT = nc.dram_tensor("xqT", [D, S], bf16, kind="ExternalInput").ap()
    xkT = nc.dram_tensor("xkT", [D, S], bf16, kind="ExternalInput").ap()
    xvT = nc.dram_tensor("xvT", [D, S], bf16, kind="ExternalInput").ap()
    wqT = nc.dram_tensor("wqT", [D, DV], bf16, kind="ExternalInput").ap()
    wkT = nc.dram_tensor("wkT", [D, DV], bf16, kind="ExternalInput").ap()
    wvT = nc.dram_tensor("wvT", [D, DV], bf16, kind="ExternalInput").ap()
    woT = nc.dram_tensor("woT", [DV, D], bf16, kind="ExternalInput").ap()
    # Output in bf16: halves the store traffic (the tail is gated by the
    # final 2MB of wo DMAs); the two head-group partials are summed in f32
    # on the host. Costs ~0.2% RMS quantization on each partial.
    out = nc.dram_tensor("out", [S, D], bf16, kind="ExternalOutput").ap()

    ND = D // 128     # 8 d-tiles
    NT = DV // 128    # 4 dk/dv-tiles
    NHP = HL // 2     # 4 head pairs

    with tile.TileContext(nc) as tc:
        with (
            tc.tile_pool(name="persist", bufs=1) as persist,
            tc.tile_pool(name="stage", bufs=6) as stage,
            tc.tile_pool(name="wpool", bufs=8) as wpool,
            tc.tile_pool(name="spool", bufs=2, space=bass.MemorySpace.PSUM) as spool,
            tc.tile_pool(name="upool", bufs=2, space=bass.MemorySpace.PSUM) as upool,
            tc.tile_pool(name="ppool", bufs=9) as ppool,
            tc.tile_pool(name="rpool", bufs=3) as rpool,
            tc.tile_pool(name="obuf", bufs=3) as obuf,
        ):
            QT = {}    # [t][qc] -> [128, 512] tiles of Q^T (dk rows, q cols)
            KT = {}    # [t][c]  -> [128, 512]
            Vaug = {}  # [kt] -> [128, 8, 65]: per-head V columns + ones col
            outT = {}  # [qc][t] -> [128, 512] normalized attention out^T
            wks, wvs, wqs, wos = [], [], [], []
            st_ = {}   # per (qc, hp) attention state
            xk_stage, xv_stage, xq_stage = {}, {}, {}
            pend = []  # scored-but-not-yet-P@V turns, FIFO

            def uacc(shape):
                return upool.tile(shape, f32, tag="u", name="uacc")

            # Bulk loads: one multi-dim-AP DMA per half-tensor instead of one
            # per 128-row tile. Trigger instructions cost ~0.7us each on the
            # sync queue, so 8-triggers-per-chunk was serializing the
            # prologue (~22us before the first exp).
            def emit_w_loads(lst, name, src, eng=None):
                eng = eng or nc.sync
                wb = wpool.tile([128, ND, DV], bf16, tag="w", name=name,
                                bufs=3)
                sv = src.rearrange("(d p) n -> p d n", p=128)
                for h in range(2):
                    eng.dma_start(wb[:, 4 * h:4 * h + 4, :],
                                  sv[:, 4 * h:4 * h + 4, :])
                lst.extend(wb[:, d, :] for d in range(ND))

            def emit_wo_loads(eng=None):
                eng = eng or nc.sync
                wb = wpool.tile([128, NT, D], bf16, tag="wo", name="wo",
                                bufs=1)
                sv = woT.rearrange("(t p) n -> p t n", p=128)
                for h in range(2):
                    eng.dma_start(wb[:, 2 * h:2 * h + 2, :],
                                  sv[:, 2 * h:2 * h + 2, :])
                wos.extend(wb[:, t, :] for t in range(NT))

            def emit_x_dmas(src, c, name, store, eng=None):
                # Prologue loads trigger from the (then-idle) scalar queue so
                # their triggers run in parallel with the w-loads on sync;
                # mid-kernel loads stay on sync (scalar is exp-saturated).
                eng = eng or nc.sync
                sv = src[:, 512 * c:512 * (c + 1)].rearrange(
                    "(d p) n -> p d n", p=128)
                xs = []
                for h in range(2):
                    xt = stage.tile([128, 4, 512], bf16, tag="act", name=name)
                    eng.dma_start(xt[:], sv[:, 4 * h:4 * h + 4, :])
                    xs.extend(xt[:, d, :] for d in range(4))
                store[c] = xs

            # Mid-kernel chunk loads ride the gpsimd software-DGE ring: a
            # third parallel stream that keeps 9MB of x-chunk traffic off
            # the two hardware queues (prologue critical path + outputs).
            # Their stage-buffer reuse waits naturally delay the transfers
            # until the prologue's critical window has drained.
            def dma_xk(c):
                return lambda: emit_x_dmas(xkT, c, "xk", xk_stage,
                                           eng=nc.gpsimd)

            def dma_xv(c):
                return lambda: emit_x_dmas(xvT, c, "xv", xv_stage,
                                           eng=nc.gpsimd)

            def dma_xq(c):
                return lambda: emit_x_dmas(xqT, c, "xq", xq_stage,
                                           eng=nc.gpsimd)

            def piece_kproj(c, t):
                def go():
                    xks = xk_stage[c]
                    acc = uacc([128, 512])
                    for d in range(ND):
                        nc.tensor.matmul(
                            acc[:], wks[d][:, 128 * t:128 * (t + 1)], xks[d][:],
                            start=(d == 0), stop=(d == ND - 1))
                    dt_ = persist.tile([128, 512], bf16, tag=f"kT{t}_{c}",
                                       name="kT")
                    nc.vector.tensor_copy(dt_[:], acc[:])
                    KT.setdefault(t, {})[c] = dt_
                return go

            def piece_vproj(c, ktl):
                # Vaug row: per head h, cols [65h, 65h+64] = V columns, col
                # 65h+64 = ones. 63 tail cols pad the last head's stationary
                # slice to 128 so every P@V stationary is [128, 128]: full
                # 128-col weights qualify for FWL / background-buffer loads,
                # which lets LDWEIGHTS overlap the previous matmul (65-col
                # stationaries were serializing, ~+100ns per P@V matmul).
                # Up rows 65..127 become garbage; folds only read rows 0..64.
                def go():
                    xvs = xv_stage[c]
                    kt = 4 * c + ktl
                    acc = uacc([128, 512])
                    for d in range(ND):
                        nc.tensor.matmul(
                            acc[:], xvs[d][:, 128 * ktl:128 * (ktl + 1)],
                            wvs[d][:],
                            start=(d == 0), stop=(d == ND - 1))
                    va = persist.tile([128, HL * (DK + 1) + 63], bf16,
                                      tag=f"vaug{kt}", name="vaug")
                    vah = va[:, 0:HL * (DK + 1)].rearrange(
                        "p (h c) -> p h c", h=HL)
                    nc.vector.tensor_copy(
                        vah[:, :, 0:DK],
                        acc[:].rearrange("p (h k) -> p h k", h=HL))
                    nc.vector.tensor_copy(
                        vah[:, :, DK], nc.const_aps.tensor(1.0, (128, HL), bf16))
                    nc.vector.tensor_copy(
                        va[:, HL * (DK + 1):],
                        nc.const_aps.tensor(1.0, (128, 63), bf16))
                    Vaug[kt] = va
                return go

            def piece_qproj(qc, t):
                def go():
                    xs = xq_stage[qc]
                    acc = uacc([128, 512])
                    for d in range(ND):
                        nc.tensor.matmul(
                            acc[:], wqs[d][:, 128 * t:128 * (t + 1)], xs[d][:],
                            start=(d == 0), stop=(d == ND - 1))
                    dt_ = persist.tile([128, 512], bf16, tag=f"qT{t}_{qc}",
                                       name="qT")
                    nc.vector.tensor_copy(dt_[:], acc[:])
                    QT.setdefault(t, {})[qc] = dt_
                return go

            def piece_wo(qc, st, ncol):
                # final[s, n] = sum_dv outT[dv, s] * woT[dv, n]
                def go():
                    acc = uacc([128, 512])
                    for t in range(NT):
                        nc.tensor.matmul(
                            acc[:],
                            outT[qc][t][:, 128 * st:128 * (st + 1)],
                            wos[t][:, 512 * ncol:512 * (ncol + 1)],
                            start=(t == 0), stop=(t == NT - 1))
                    ob = obuf.tile([128, 512], bf16, tag="ob", name="ob",
                                   bufs=2)
                    nc.vector.tensor_copy(ob[:], acc[:])
                    nc.sync.dma_start(
                        out[512 * qc + 128 * st:512 * qc + 128 * (st + 1),
                            512 * ncol:512 * (ncol + 1)],
                        ob[:])
                return go

            # wo for the last q-chunk is split into halves so the first half
            # (heads 0-3, t=0,1) runs while heads 4-7 are still attending;
            # only the second half + combine drains in the tail.
            obp = {}

            def piece_wo_ab(qc, st, ncol):
                def go():
                    acc = uacc([128, 512])
                    for t in range(2):
                        nc.tensor.matmul(
                            acc[:],
                            outT[qc][t][:, 128 * st:128 * (st + 1)],
                            wos[t][:, 512 * ncol:512 * (ncol + 1)],
                            start=(t == 0), stop=(t == 1))
                    pb = obuf.tile([128, 512], bf16, tag="obp", name="obp",
                                   bufs=8)
                    nc.vector.tensor_copy(pb[:], acc[:])
                    obp[(st, ncol)] = pb
                return go

            def piece_wo_cd(qc, st, ncol):
                # tail-only: borrow the (idle) score PSUM banks so three
                # accumulators rotate and the matmuls never wait on the
                # combine adds.
                def go():
                    acc = spool.tile([128, 1536], f32, tag="sc",
                                     name="sc")[:, 0:512]
                    for t in range(2, NT):
                        nc.tensor.matmul(
                            acc[:],
                            outT[qc][t][:, 128 * st:128 * (st + 1)],
                            wos[t][:, 512 * ncol:512 * (ncol + 1)],
                            start=(t == 2), stop=(t == NT - 1))
                    ob = obuf.tile([128, 512], bf16, tag="ob", name="ob",
                                   bufs=2)
                    nc.vector.tensor_add(ob[:], obp[(st, ncol)][:], acc[:])
                    # tail-only: the scalar queue is idle once the exps are
                    # done; keep the final stores off the busy sync queue.
                    nc.scalar.dma_start(
                        out[512 * qc + 128 * st:512 * qc + 128 * (st + 1),
                            512 * ncol:512 * (ncol + 1)],
                        ob[:])
                return go

            # Scores stream through a rolling ring of [128, 1536] PSUM tiles
            # (3-bank, double-buffered = same 3072-element exp lookahead as
            # 3 x [128,1024], but each ACTIVATE covers three 512-blocks,
            # amortizing the ~260ns per-instruction overhead). Block
            # placement is decoupled from head/kp boundaries; P@V finds each
            # block's post-exp slice via Pslices.
            cur = {"sc": None, "w": 0, "keys": []}
            Pslices = {}

            def put_block(key):
                if cur["sc"] is None:
                    cur["sc"] = spool.tile([128, 1536], f32, tag="sc",
                                           name="sc")
                    cur["w"] = 0
                    cur["keys"] = []
                ap = cur["sc"][:, 512 * cur["w"]:512 * (cur["w"] + 1)]
                cur["keys"].append(key)
                cur["w"] += 1
                return ap

            def flush_exp(force=False):
                if cur["sc"] is None or (cur["w"] < 3 and not force):
                    return
                w = cur["w"]
                Pt = ppool.tile([128, 1536], bf16, tag="p", name="p")
                nc.scalar.activation(Pt[:, 0:512 * w], cur["sc"][:, 0:512 * w],
                                     EXP, scale=0.125)
                for idx, key in enumerate(cur["keys"]):
                    Pslices[key] = Pt[:, 512 * idx:512 * (idx + 1)]
                cur["sc"] = None

            def emit_scores_half(qc, hp, kp):
                # one kp-block: row-tiled score matmul pair for both heads.
                t = hp
                for j in range(2):
                    kt = 2 * kp + j
                    for i in range(2):
                        po = 64 * i
                        ap = put_block((qc, hp, kp, i, j))
                        nc.tensor.matmul(
                            ap,
                            KT[t][kt // 4][po:po + 64,
                                           128 * (kt % 4):128 * (kt % 4 + 1)],
                            QT[t][qc][po:po + 64, :],
                            start=True, stop=True)
                        flush_exp()

            def emit_pv(ent):
                # P@V + denominator for one (qc, hp, c) turn: per head, one
                # 4-k-tile PSUM accumulation folded into the SBUF Usb.
                qc, hp, c = ent
                t = hp
                s = st_.setdefault((qc, hp), {})
                if c == 0:
                    s["Usb"] = [rpool.tile([65, 512], f32, tag=f"usb{hp}_{i}",
                                           name="usb", bufs=2)
                                for i in range(2)]
                for i in range(2):
                    i2 = 2 * hp + i
                    Up = uacc([128, 512])
                    for kp in (2 * c, 2 * c + 1):
                        for j in range(2):
                            kt = 2 * kp + j
                            nc.tensor.matmul(
                                Up[:],
                                Vaug[kt][:, 65 * i2:65 * i2 + 128],
                                Pslices.pop((qc, hp, kp, i, j)),
                                start=(kt % 4 == 0), stop=(kt % 4 == 3))
                    if c == 0:
                        nc.vector.tensor_copy(s["Usb"][i][:], Up[0:65, :])
                    else:
                        nc.vector.tensor_add(s["Usb"][i][:],
                                             s["Usb"][i][:], Up[0:65, :])
                if c == 3:
                    if in_tail[0]:
                        emit_normalize_fast(qc, hp)
                    else:
                        emit_normalize(qc, hp)

            def emit_normalize(qc, hp):
                # rows 0..63 of U divided by row 64 (the ones-column sum),
                # written into out^T. Engine ops keep operands on one
                # partition range; cross-partition moves via SBUF-SBUF DMA.
                t = hp
                Usb = st_[(qc, hp)]["Usb"]
                ot = persist.tile([128, 512], bf16, tag=f"oT{t}_{qc % 2}",
                                  name="oT")
                outT.setdefault(qc, {})[t] = ot
                for i in range(2):
                    rrow = rpool.tile([1, 512], f32, tag="rrow", name="rrow")
                    nc.sync.dma_start(rrow[:], Usb[i][64:65, :])
                    rrec = rpool.tile([1, 512], f32, tag="rrec", name="rrec")
                    nc.vector.reciprocal_approx_fast(rrec[:], rrow[:])
                    rb = rpool.tile([64, 512], f32, tag="rb", name="rb")
                    nc.gpsimd.partition_broadcast(rb[:], rrec[:])
                    if i == 0:
                        nc.vector.tensor_mul(ot[0:64, :], Usb[i][0:64, :],
                                             rb[:])
                    else:
                        stg = rpool.tile([64, 512], bf16, tag="stg",
                                         name="stg")
                        nc.vector.tensor_mul(stg[:], Usb[i][0:64, :], rb[:])
                        nc.sync.dma_start(ot[64:128, :], stg[:])

            ones65 = persist.tile([65, 64], bf16, tag="ones65", name="ones65")

            def emit_normalize_fast(qc, hp):
                # Tail-only: replaces the sync-DMA + gpsimd-broadcast chain
                # (~2.5us of cross-engine latency gating every tail wo piece)
                # with a PE outer product from a real partition-64 ones row.
                # bf16 reciprocal costs ~0.4% on these rows' denominators.
                t = hp
                Usb = st_[(qc, hp)]["Usb"]
                ot = persist.tile([128, 512], bf16, tag=f"oT{t}_{qc % 2}",
                                  name="oT")
                outT.setdefault(qc, {})[t] = ot
                for i in range(2):
                    rf = rpool.tile([65, 512], f32, tag="rb", name="rrf")
                    nc.vector.reciprocal_approx_fast(rf[64:65, :],
                                                     Usb[i][64:65, :])
                    rr = rpool.tile([65, 512], bf16, tag="stg", name="rrb")
                    nc.vector.tensor_copy(rr[64:65, :], rf[64:65, :])
                    rb = uacc([128, 512])
                    nc.tensor.matmul(rb[0:64, :], ones65[64:65, :],
                                     rr[64:65, :], start=True, stop=True)
                    if i == 0:
                        nc.vector.tensor_mul(ot[0:64, :], Usb[i][0:64, :],
                                             rb[0:64, :])
                    else:
                        stg = rpool.tile([64, 512], bf16, tag="stg",
                                         name="stg")
                        nc.vector.tensor_mul(stg[:], Usb[i][0:64, :],
                                             rb[0:64, :])
                        nc.sync.dma_start(ot[64:128, :], stg[:])

            in_tail = [False]

            def vaug_ready(c):
                return all((4 * c + k) in Vaug for k in range(4))

            def drain_pv():
                if pend and vaug_ready(pend[0][2]):
                    qc, hp, c = pend[0]
                    if all((qc, hp, kp, i, j) in Pslices
                           for kp in (2 * c, 2 * c + 1)
                           for i in range(2) for j in range(2)):
                        emit_pv(pend.pop(0))

            def turn(qc, hp, c, acts):
                # P@V of the *previous* turn is emitted after this turn's
                # scores+pieces, so its exp is already done when the PE
                # reaches it (drain before append = one-turn lag).
                emit_scores_half(qc, hp, 2 * c)
                if len(acts) > 0:
                    acts[0]()
                emit_scores_half(qc, hp, 2 * c + 1)
                if len(acts) > 1:
                    acts[1]()
                drain_pv()
                pend.append((qc, hp, c))
                for a in acts[2:]:
                    a()

            # ---- DMA prologue: K and Q paths first (needed first), then V,
            # then wo and the second q-chunk's activations.
            # Three parallel trigger/queue streams; sync and scalar (the two
            # hardware-DGE queues) carry only the first-exp critical path
            # (K and Q), everything else rides gpsimd software DGE.
            # Both HW-DGE queues carry the critical K/Q path first; V and the
            # rest queue strictly behind on the same engines so their bytes
            # don't steal HBM bandwidth from the first-exp critical path.
            emit_w_loads(wks, "wk", wkT)
            emit_x_dmas(xkT, 0, "xk", xk_stage, eng=nc.scalar)
            emit_w_loads(wqs, "wq", wqT)
            emit_x_dmas(xqT, 0, "xq", xq_stage, eng=nc.scalar)
            emit_w_loads(wvs, "wv", wvT)
            emit_x_dmas(xvT, 0, "xv", xv_stage, eng=nc.scalar)
            emit_x_dmas(xqT, 1, "xq", xq_stage, eng=nc.scalar)
            emit_wo_loads()

            # ---- prologue pieces: all of K0 runs in the shadow of the
            # Q-path DMAs (K-path data lands ~8us earlier), then Q0t0 gates
            # the first scores. V0 lands in turns 0-2 so the P@V pending
            # queue drains with lag 2 from turn 2 on.
            nc.vector.tensor_copy(ones65[64:65, :],
                                  nc.const_aps.tensor(1.0, (1, 64), bf16))
            for t in range(NT):
                piece_kproj(0, t)()
            piece_qproj(0, 0)()

            # ---- per-turn piece schedule ----
            # Super-round 01 interleaves qc0+qc1 (c, qc, hp); qc2 and qc3
            # follow (c, hp). JIT rule: a piece emitted in turn T is usable
            # from turn T+1 on.
            sched = {
                0: [piece_qproj(0, 1), piece_vproj(0, 0)],
                1: [piece_qproj(0, 2), piece_vproj(0, 1)],
                2: [piece_vproj(0, 2), piece_vproj(0, 3), piece_qproj(0, 3)],
                3: [dma_xk(1), piece_qproj(1, 0)],
                4: [piece_qproj(1, 1), piece_kproj(1, 0)],
                5: [piece_qproj(1, 2), piece_kproj(1, 1)],
                6: [piece_qproj(1, 3), piece_kproj(1, 2)],
                7: [piece_kproj(1, 3), dma_xv(1)],
                8: [dma_xk(2), piece_vproj(1, 0), piece_vproj(1, 1)],
                9: [piece_vproj(1, 2), piece_vproj(1, 3)],
                10: [piece_kproj(2, 0), dma_xv(2)],
                11: [piece_kproj(2, 1)],
                12: [piece_kproj(2, 2)],
                13: [piece_kproj(2, 3), dma_xk(3)],
                14: [piece_vproj(2, 0)],
                15: [piece_vproj(2, 1)],
                16: [piece_vproj(2, 2), dma_xv(3)],
                17: [piece_vproj(2, 3)],
                18: [piece_kproj(3, 0), dma_xq(2)],
                19: [piece_kproj(3, 1)],
                20: [piece_kproj(3, 2)],
                21: [piece_kproj(3, 3)],
                22: [piece_vproj(3, 0)],
                23: [piece_vproj(3, 1)],
                24: [piece_vproj(3, 2)],
                25: [piece_vproj(3, 3)],
                26: [piece_qproj(2, 0)],
                27: [piece_qproj(2, 1)],
                28: [piece_qproj(2, 2)],
                29: [piece_qproj(2, 3)],
                30: [dma_xq(3)],
                32: [piece_qproj(3, 0)],
                33: [piece_qproj(3, 1)],
                34: [piece_qproj(3, 2)],
                35: [piece_qproj(3, 3)],
            }
            for k in range(8):
                sched[36 + k] = [piece_wo(0, k // 2, k % 2)]
                sched[44 + k] = [piece_wo(1, k // 2, k % 2)]
                sched[52 + k] = [piece_wo(2, k // 2, k % 2)]
            for k in range(4):
                sched[60 + k] = [piece_wo_ab(3, k, 0), piece_wo_ab(3, k, 1)]

            turns = []
            for c in range(4):
                for g in range(2):
                    for hp in range(NHP):
                        turns.append((g, hp, c))
            for c in range(4):
                for hp in range(NHP):
                    turns.append((2, hp, c))
            # qc3 runs hp-outer so heads finish (and normalize) early enough
            # for the first wo half to overlap the last head's attention.
            for hp in range(NHP):
                for c in range(4):
                    turns.append((3, hp, c))
            assert len(turns) == 64

            for ti, (qc, hp, c) in enumerate(turns):
                turn(qc, hp, c, sched.pop(ti, []))
            assert not sched, sched

            flush_exp(force=True)
            while pend:
                emit_pv(pend.pop(0))
            for k in range(8):
                piece_wo_cd(3, k // 2, k % 2)()

    nc.compile()
    return nc


def kernel(query, key, value, w_q, w_k, w_v, w_o):
    import ml_dtypes
    from concourse.bass_utils import run_bass_kernel_spmd

    if "nc" not in _cached:
        _cached["nc"] = _build()
    nc = _cached["nc"]

    bf = ml_dtypes.bfloat16
    query = np.asarray(query, dtype=np.float32)
    key = np.asarray(key, dtype=np.float32)
    value = np.asarray(value, dtype=np.float32)
    w_q = np.asarray(w_q, dtype=np.float32)
    w_k = np.asarray(w_k, dtype=np.float32)
    w_v = np.asarray(w_v, dtype=np.float32)
    w_o = np.asarray(w_o, dtype=np.float32)

    def c(a):
        return np.ascontiguousarray(a).astype(bf)

    in_maps = []
    for core in range(N_CORES):
        b, g = core // G, core % G
        rows = slice(DV * g, DV * (g + 1))
        in_maps.append({
            "xqT": c(query[b].T),
            "xkT": c(key[b].T),
            "xvT": c(value[b].T),
            "wqT": c(w_q[rows, :].T),
            "wkT": c(w_k[rows, :].T),
            "wvT": c(w_v[rows, :].T),
            "woT": c(w_o[:, rows].T),
        })

    res = run_bass_kernel_spmd(nc, in_maps, list(range(N_CORES)))
    full = np.empty((B, S, D), np.float32)
    for b in range(B):
        full[b] = (res.results[G * b]["out"].astype(np.float32)
                   + res.results[G * b + 1]["out"].astype(np.float32))
    return full


# revision 70
# speedup vs baseline: 1.0429x; 1.0429x over previous
"""Multi-head attention (B=4, S=2048, D=1024, H=16) on 8 TRN2 NeuronCores.

Sharding: data-parallel over batch (4) x tensor-parallel over head halves (2).
Core c handles batch b = c//2 and heads [8g, 8g+8) where g = c%2.
Each core computes a partial [S, D] output-projection contribution; the host
sums the two head-group partials per batch.

All activations are passed to the device pre-transposed (feature dim on
partitions) so the kernel needs no on-device transposes:
  - projections contract over d (model dim) with host-transposed x^T,
  - scores are built transposed [k, q] so exp() output feeds the P @ V
    matmul directly as the moving operand,
  - P @ [V | 1] yields the softmax denominator as row 64 of U^T for free,
  - normalized U^T tiles are exactly the stationary layout w_o needs.

Schedule (v2): the scalar engine's exp stream (256 x [128,1024] ACTIVATEs at
~1.34us each = ~342us) is the hard floor; everything else is arranged to keep
both ACT and PE dense:
  - q-chunks 0 and 1 are interleaved into one 32-turn super-round (c-outer,
    then qc, then hp) so the K/V/Q projection pieces spread at <=2 per turn
    instead of all landing in q-chunk 0.
  - P@V for a turn is emitted one turn later (pending queue), so its exp
    dependency is already satisfied and never head-of-line-blocks the
    in-order PE stream; this also keeps the PE queue backed up so the
    row-tiled score matmul pairs co-start (co-execution needs both
    LDWEIGHTS done before the first MM issues).
  - projection/wo pieces are emitted *inside* turns (between the two
    kp-blocks and after them), never in a burst.
  - wo(qc0) runs during qc2's turns, wo(qc1)+wo(qc2) during qc3's; only
    wo(qc3) (8 pieces) drains in the tail.

Matmul operands are bf16 (fp32 PSUM accumulation). Head pairs share the PE
array via row tiling (partitions 0-63 / 64-127) so the DK=64 score matmuls
run concurrently when co-issued.

PSUM budget (8 banks): 3 x [128,1024] score tiles (6 banks) + one shared
2-slot pool (2 banks) for every [<=128,512] accumulator; attention U
accumulates in SBUF via DVE adds of 4-k-tile PSUM partials so no PSUM slot
is held for long.
"""

import numpy as np

B, S, D, H = 4, 2048, 1024, 16
DK = D // H          # 64
G = 2                # head groups (tensor-parallel degree per batch)
HL = H // G          # 8 local heads per core
DV = HL * DK         # 512 local value dim
N_CORES = 8

_cached = {}


def _build():
    import concourse.bass as bass
    import concourse.tile as tile
    from concourse import bacc, mybir

    f32 = mybir.dt.float32
    bf16 = mybir.dt.bfloat16
    EXP = mybir.ActivationFunctionType.Exp

    nc = bacc.Bacc("TRN2", target_bir_lowering=False, debug=False,
                   num_devices=N_CORES)

    xqT = nc.dram_tensor("xqT", [D, S], bf16, kind="ExternalInput").ap()
    xkT = nc.dram_tensor("xkT", [D, S], bf16, kind="ExternalInput").ap()
    xvT = nc.dram_tensor("xvT", [D, S], bf16, kind="ExternalInput").ap()
    wqT = nc.dram_tensor("wqT", [D, DV], bf16, kind="ExternalInput").ap()
    wkT = nc.dram_tensor("wkT", [D, DV], bf16, kind="ExternalInput").ap()
    wvT = nc.dram_tensor("wvT", [D, DV], bf16, kind="ExternalInput").ap()
    woT = nc.dram_tensor("woT", [DV, D], bf16, kind="ExternalInput").ap()
    # Output in bf16: halves the store traffic (the tail is gated by the
    # final 2MB of wo DMAs); the two head-group partials are summed in f32
    # on the host. Costs ~0.2% RMS quantization on each partial.
    out = nc.dram_tensor("out", [S, D], bf16, kind="ExternalOutput").ap()

    ND = D // 128     # 8 d-tiles
    NT = DV // 128    # 4 dk/dv-tiles
    NHP = HL // 2     # 4 head pairs

    with tile.TileContext(nc) as tc:
        with (
            tc.tile_pool(name="persist", bufs=1) as persist,
            tc.tile_pool(name="stage", bufs=6) as stage,
            tc.tile_pool(name="wpool", bufs=8) as wpool,
            tc.tile_pool(name="spool", bufs=3, space=bass.MemorySpace.PSUM) as spool,
            tc.tile_pool(name="upool", bufs=2, space=bass.MemorySpace.PSUM) as upool,
            tc.tile_pool(name="ppool", bufs=14) as ppool,
            tc.tile_pool(name="rpool", bufs=3) as rpool,
            tc.tile_pool(name="obuf", bufs=3) as obuf,
        ):
            QT = {}    # [t][qc] -> [128, 512] tiles of Q^T (dk rows, q cols)
            KT = {}    # [t][c]  -> [128, 512]
            Vaug = {}  # [kt] -> [128, 8, 65]: per-head V columns + ones col
            outT = {}  # [qc][t] -> [128, 512] normalized attention out^T
            wks, wvs, wqs, wos = [], [], [], []
            st_ = {}   # per (qc, hp) attention state
            xk_stage, xv_stage, xq_stage = {}, {}, {}
            pend = []  # scored-but-not-yet-P@V turns, FIFO

            def uacc(shape):
                return upool.tile(shape, f32, tag="u", name="uacc")

            # Bulk loads: one multi-dim-AP DMA per half-tensor instead of one
            # per 128-row tile. Trigger instructions cost ~0.7us each on the
            # sync queue, so 8-triggers-per-chunk was serializing the
            # prologue (~22us before the first exp).
            def emit_w_loads(lst, name, src, eng=None):
                eng = eng or nc.sync
                wb = wpool.tile([128, ND, DV], bf16, tag="w", name=name,
                                bufs=3)
                sv = src.rearrange("(d p) n -> p d n", p=128)
                for h in range(2):
                    eng.dma_start(wb[:, 4 * h:4 * h + 4, :],
                                  sv[:, 4 * h:4 * h + 4, :])
                lst.extend(wb[:, d, :] for d in range(ND))

            def emit_wo_loads(eng=None):
                eng = eng or nc.sync
                wb = wpool.tile([128, NT, D], bf16, tag="wo", name="wo",
                                bufs=1)
                sv = woT.rearrange("(t p) n -> p t n", p=128)
                for h in range(2):
                    eng.dma_start(wb[:, 2 * h:2 * h + 2, :],
                                  sv[:, 2 * h:2 * h + 2, :])
                wos.extend(wb[:, t, :] for t in range(NT))

            def emit_x_dmas(src, c, name, store, eng=None):
                # Prologue loads trigger from the (then-idle) scalar queue so
                # their triggers run in parallel with the w-loads on sync;
                # mid-kernel loads stay on sync (scalar is exp-saturated).
                eng = eng or nc.sync
                sv = src[:, 512 * c:512 * (c + 1)].rearrange(
                    "(d p) n -> p d n", p=128)
                xs = []
                for h in range(2):
                    xt = stage.tile([128, 4, 512], bf16, tag="act", name=name)
                    eng.dma_start(xt[:], sv[:, 4 * h:4 * h + 4, :])
                    xs.extend(xt[:, d, :] for d in range(4))
                store[c] = xs

            # Mid-kernel chunk loads ride the gpsimd software-DGE ring: a
            # third parallel stream that keeps 9MB of x-chunk traffic off
            # the two hardware queues (prologue critical path + outputs).
            # Their stage-buffer reuse waits naturally delay the transfers
            # until the prologue's critical window has drained.
            def dma_xk(c):
                return lambda: emit_x_dmas(xkT, c, "xk", xk_stage,
                                           eng=nc.gpsimd)

            def dma_xv(c):
                return lambda: emit_x_dmas(xvT, c, "xv", xv_stage,
                                           eng=nc.gpsimd)

            def dma_xq(c):
                return lambda: emit_x_dmas(xqT, c, "xq", xq_stage,
                                           eng=nc.gpsimd)

            def piece_kproj(c, t):
                def go():
                    xks = xk_stage[c]
                    acc = uacc([128, 512])
                    for d in range(ND):
                        nc.tensor.matmul(
                            acc[:], wks[d][:, 128 * t:128 * (t + 1)], xks[d][:],
                            start=(d == 0), stop=(d == ND - 1))
                    dt_ = persist.tile([128, 512], bf16, tag=f"kT{t}_{c}",
                                       name="kT")
                    nc.vector.tensor_copy(dt_[:], acc[:])
                    KT.setdefault(t, {})[c] = dt_
                return go

            def piece_vproj(c, ktl):
                # Vaug row: per head h, cols [65h, 65h+64] = V columns, col
                # 65h+64 = ones. 63 tail cols pad the last head's stationary
                # slice to 128 so every P@V stationary is [128, 128]: full
                # 128-col weights qualify for FWL / background-buffer loads,
                # which lets LDWEIGHTS overlap the previous matmul (65-col
                # stationaries were serializing, ~+100ns per P@V matmul).
                # Up rows 65..127 become garbage; folds only read rows 0..64.
                def go():
                    xvs = xv_stage[c]
                    kt = 4 * c + ktl
                    acc = uacc([128, 512])
                    for d in range(ND):
                        nc.tensor.matmul(
                            acc[:], xvs[d][:, 128 * ktl:128 * (ktl + 1)],
                            wvs[d][:],
                            start=(d == 0), stop=(d == ND - 1))
                    va = persist.tile([128, HL * (DK + 1) + 63], bf16,
                                      tag=f"vaug{kt}", name="vaug")
                    vah = va[:, 0:HL * (DK + 1)].rearrange(
                        "p (h c) -> p h c", h=HL)
                    nc.vector.tensor_copy(
                        vah[:, :, 0:DK],
                        acc[:].rearrange("p (h k) -> p h k", h=HL))
                    nc.vector.tensor_copy(
                        vah[:, :, DK], nc.const_aps.tensor(1.0, (128, HL), bf16))
                    nc.vector.tensor_copy(
                        va[:, HL * (DK + 1):],
                        nc.const_aps.tensor(1.0, (128, 63), bf16))
                    Vaug[kt] = va
                return go

            def piece_qproj(qc, t):
                def go():
                    xs = xq_stage[qc]
                    acc = uacc([128, 512])
                    for d in range(ND):
                        nc.tensor.matmul(
                            acc[:], wqs[d][:, 128 * t:128 * (t + 1)], xs[d][:],
                            start=(d == 0), stop=(d == ND - 1))
                    dt_ = persist.tile([128, 512], bf16, tag=f"qT{t}_{qc}",
                                       name="qT")
                    nc.vector.tensor_copy(dt_[:], acc[:])
                    QT.setdefault(t, {})[qc] = dt_
                return go

            def piece_wo(qc, st, ncol):
                # final[s, n] = sum_dv outT[dv, s] * woT[dv, n]
                def go():
                    acc = uacc([128, 512])
                    for t in range(NT):
                        nc.tensor.matmul(
                            acc[:],
                            outT[qc][t][:, 128 * st:128 * (st + 1)],
                            wos[t][:, 512 * ncol:512 * (ncol + 1)],
                            start=(t == 0), stop=(t == NT - 1))
                    ob = obuf.tile([128, 512], bf16, tag="ob", name="ob",
                                   bufs=2)
                    nc.vector.tensor_copy(ob[:], acc[:])
                    nc.sync.dma_start(
                        out[512 * qc + 128 * st:512 * qc + 128 * (st + 1),
                            512 * ncol:512 * (ncol + 1)],
                        ob[:])
                return go

            # wo for the last q-chunk is split into halves so the first half
            # (heads 0-3, t=0,1) runs while heads 4-7 are still attending;
            # only the second half + combine drains in the tail.
            obp = {}

            def piece_wo_ab(qc, st, ncol):
                def go():
                    acc = uacc([128, 512])
                    for t in range(2):
                        nc.tensor.matmul(
                            acc[:],
                            outT[qc][t][:, 128 * st:128 * (st + 1)],
                            wos[t][:, 512 * ncol:512 * (ncol + 1)],
                            start=(t == 0), stop=(t == 1))
                    pb = obuf.tile([128, 512], bf16, tag="obp", name="obp",
                                   bufs=8)
                    nc.vector.tensor_copy(pb[:], acc[:])
                    obp[(st, ncol)] = pb
                return go

            def piece_wo_cd(qc, st, ncol):
                # tail-only: borrow the (idle) score PSUM banks so three
                # accumulators rotate and the matmuls never wait on the
                # combine adds.
                def go():
                    acc = spool.tile([128, 1024], f32, tag="sc",
                                     name="sc")[:, 0:512]
                    for t in range(2, NT):
                        nc.tensor.matmul(
                            acc[:],
                            outT[qc][t][:, 128 * st:128 * (st + 1)],
                            wos[t][:, 512 * ncol:512 * (ncol + 1)],
                            start=(t == 2), stop=(t == NT - 1))
                    ob = obuf.tile([128, 512], bf16, tag="ob", name="ob",
                                   bufs=2)
                    nc.vector.tensor_add(ob[:], obp[(st, ncol)][:], acc[:])
                    # tail-only: the scalar queue is idle once the exps are
                    # done; keep the final stores off the busy sync queue.
                    nc.scalar.dma_start(
                        out[512 * qc + 128 * st:512 * qc + 128 * (st + 1),
                            512 * ncol:512 * (ncol + 1)],
                        ob[:])
                return go

            def emit_scores_half(qc, hp, kp):
                # one kp-block: row-tiled score matmul pair for both heads of
                # the pair, then their exps. Returns {(kp, i): P tile}.
                t = hp
                sc = [spool.tile([128, 1024], f32, tag="sc", name="sc")
                      for _ in range(2)]
                for j in range(2):
                    kt = 2 * kp + j
                    for i in range(2):
                        po = 64 * i
                        nc.tensor.matmul(
                            sc[i][:, 512 * j:512 * (j + 1)],
                            KT[t][kt // 4][po:po + 64,
                                           128 * (kt % 4):128 * (kt % 4 + 1)],
                            QT[t][qc][po:po + 64, :],
                            start=True, stop=True)
                Ps = {}
                for i in range(2):
                    P = ppool.tile([128, 1024], bf16, tag="p", name="p")
                    nc.scalar.activation(P[:], sc[i][:], EXP, scale=0.125)
                    Ps[(kp, i)] = P
                return Ps

            def emit_pv(ent):
                # P@V + denominator for one (qc, hp, c) turn: per head, one
                # 4-k-tile PSUM accumulation folded into the SBUF Usb.
                qc, hp, c, Ps = ent
                t = hp
                s = st_.setdefault((qc, hp), {})
                if c == 0:
                    s["Usb"] = [rpool.tile([65, 512], f32, tag=f"usb{hp}_{i}",
                                           name="usb", bufs=2)
                                for i in range(2)]
                for i in range(2):
                    i2 = 2 * hp + i
                    Up = uacc([128, 512])
                    for kp in (2 * c, 2 * c + 1):
                        for j in range(2):
                            kt = 2 * kp + j
                            nc.tensor.matmul(
                                Up[:],
                                Vaug[kt][:, 65 * i2:65 * i2 + 128],
                                Ps[(kp, i)][:, 512 * j:512 * (j + 1)],
                                start=(kt % 4 == 0), stop=(kt % 4 == 3))
                    if c == 0:
                        nc.vector.tensor_copy(s["Usb"][i][:], Up[0:65, :])
                    else:
                        nc.vector.tensor_add(s["Usb"][i][:],
                                             s["Usb"][i][:], Up[0:65, :])
                if c == 3:
                    if in_tail[0]:
                        emit_normalize_fast(qc, hp)
                    else:
                        emit_normalize(qc, hp)

            def emit_normalize(qc, hp):
                # rows 0..63 of U divided by row 64 (the ones-column sum),
                # written into out^T. Engine ops keep operands on one
                # partition range; cross-partition moves via SBUF-SBUF DMA.
                t = hp
                Usb = st_[(qc, hp)]["Usb"]
                ot = persist.tile([128, 512], bf16, tag=f"oT{t}_{qc % 2}",
                                  name="oT")
                outT.setdefault(qc, {})[t] = ot
                for i in range(2):
                    rrow = rpool.tile([1, 512], f32, tag="rrow", name="rrow")
                    nc.sync.dma_start(rrow[:], Usb[i][64:65, :])
                    rrec = rpool.tile([1, 512], f32, tag="rrec", name="rrec")
                    nc.vector.reciprocal_approx_fast(rrec[:], rrow[:])
                    rb = rpool.tile([64, 512], f32, tag="rb", name="rb")
                    nc.gpsimd.partition_broadcast(rb[:], rrec[:])
                    if i == 0:
                        nc.vector.tensor_mul(ot[0:64, :], Usb[i][0:64, :],
                                             rb[:])
                    else:
                        stg = rpool.tile([64, 512], bf16, tag="stg",
                                         name="stg")
                        nc.vector.tensor_mul(stg[:], Usb[i][0:64, :], rb[:])
                        nc.sync.dma_start(ot[64:128, :], stg[:])

            ones65 = persist.tile([65, 64], bf16, tag="ones65", name="ones65")

            def emit_normalize_fast(qc, hp):
                # Tail-only: replaces the sync-DMA + gpsimd-broadcast chain
                # (~2.5us of cross-engine latency gating every tail wo piece)
                # with a PE outer product from a real partition-64 ones row.
                # bf16 reciprocal costs ~0.4% on these rows' denominators.
                t = hp
                Usb = st_[(qc, hp)]["Usb"]
                ot = persist.tile([128, 512], bf16, tag=f"oT{t}_{qc % 2}",
                                  name="oT")
                outT.setdefault(qc, {})[t] = ot
                for i in range(2):
                    rf = rpool.tile([65, 512], f32, tag="rb", name="rrf")
                    nc.vector.reciprocal_approx_fast(rf[64:65, :],
                                                     Usb[i][64:65, :])
                    rr = rpool.tile([65, 512], bf16, tag="stg", name="rrb")
                    nc.vector.tensor_copy(rr[64:65, :], rf[64:65, :])
                    rb = uacc([128, 512])
                    nc.tensor.matmul(rb[0:64, :], ones65[64:65, :],
                                     rr[64:65, :], start=True, stop=True)
                    if i == 0:
                        nc.vector.tensor_mul(ot[0:64, :], Usb[i][0:64, :],
                                             rb[0:64, :])
                    else:
                        stg = rpool.tile([64, 512], bf16, tag="stg",
                                         name="stg")
                        nc.vector.tensor_mul(stg[:], Usb[i][0:64, :],
                                             rb[0:64, :])
                        nc.sync.dma_start(ot[64:128, :], stg[:])

            in_tail = [False]

            def vaug_ready(c):
                return all((4 * c + k) in Vaug for k in range(4))

            def drain_pv():
                if pend and vaug_ready(pend[0][2]):
                    emit_pv(pend.pop(0))

            def turn(qc, hp, c, acts):
                # P@V of the *previous* turn is emitted after this turn's
                # scores+pieces, so its exp is already done when the PE
                # reaches it (drain before append = one-turn lag).
                p0 = emit_scores_half(qc, hp, 2 * c)
                if len(acts) > 0:
                    acts[0]()
                p1 = emit_scores_half(qc, hp, 2 * c + 1)
                if len(acts) > 1:
                    acts[1]()
                drain_pv()
                pend.append((qc, hp, c, {**p0, **p1}))
                for a in acts[2:]:
                    a()

            # ---- DMA prologue: K and Q paths first (needed first), then V,
            # then wo and the second q-chunk's activations.
            # Three parallel trigger/queue streams; sync and scalar (the two
            # hardware-DGE queues) carry only the first-exp critical path
            # (K and Q), everything else rides gpsimd software DGE.
            # Both HW-DGE queues carry the critical K/Q path first; V and the
            # rest queue strictly behind on the same engines so their bytes
            # don't steal HBM bandwidth from the first-exp critical path.
            emit_w_loads(wks, "wk", wkT)
            emit_x_dmas(xkT, 0, "xk", xk_stage, eng=nc.scalar)
            emit_w_loads(wqs, "wq", wqT)
            emit_x_dmas(xqT, 0, "xq", xq_stage, eng=nc.scalar)
            emit_w_loads(wvs, "wv", wvT)
            emit_x_dmas(xvT, 0, "xv", xv_stage, eng=nc.scalar)
            emit_x_dmas(xqT, 1, "xq", xq_stage, eng=nc.scalar)
            emit_wo_loads()

            # ---- prologue pieces: all of K0 runs in the shadow of the
            # Q-path DMAs (K-path data lands ~8us earlier), then Q0t0 gates
            # the first scores. V0 lands in turns 0-2 so the P@V pending
            # queue drains with lag 2 from turn 2 on.
            nc.vector.tensor_copy(ones65[64:65, :],
                                  nc.const_aps.tensor(1.0, (1, 64), bf16))
            for t in range(NT):
                piece_kproj(0, t)()
            piece_qproj(0, 0)()

            # ---- per-turn piece schedule ----
            # Super-round 01 interleaves qc0+qc1 (c, qc, hp); qc2 and qc3
            # follow (c, hp). JIT rule: a piece emitted in turn T is usable
            # from turn T+1 on.
            sched = {
                0: [piece_qproj(0, 1), piece_vproj(0, 0)],
                1: [piece_qproj(0, 2), piece_vproj(0, 1)],
                2: [piece_vproj(0, 2), piece_vproj(0, 3), piece_qproj(0, 3)],
                3: [dma_xk(1), piece_qproj(1, 0)],
                4: [piece_qproj(1, 1), piece_kproj(1, 0)],
                5: [piece_qproj(1, 2), piece_kproj(1, 1)],
                6: [piece_qproj(1, 3), piece_kproj(1, 2)],
                7: [piece_kproj(1, 3), dma_xv(1)],
                8: [dma_xk(2), piece_vproj(1, 0), piece_vproj(1, 1)],
                9: [piece_vproj(1, 2), piece_vproj(1, 3)],
                10: [piece_kproj(2, 0), dma_xv(2)],
                11: [piece_kproj(2, 1)],
                12: [piece_kproj(2, 2)],
                13: [piece_kproj(2, 3), dma_xk(3)],
                14: [piece_vproj(2, 0)],
                15: [piece_vproj(2, 1)],
                16: [piece_vproj(2, 2), dma_xv(3)],
                17: [piece_vproj(2, 3)],
                18: [piece_kproj(3, 0), dma_xq(2)],
                19: [piece_kproj(3, 1)],
                20: [piece_kproj(3, 2)],
                21: [piece_kproj(3, 3)],
                22: [piece_vproj(3, 0)],
                23: [piece_vproj(3, 1)],
                24: [piece_vproj(3, 2)],
                25: [piece_vproj(3, 3)],
                26: [piece_qproj(2, 0)],
                27: [piece_qproj(2, 1)],
                28: [piece_qproj(2, 2)],
                29: [piece_qproj(2, 3)],
                30: [dma_xq(3)],
                32: [piece_qproj(3, 0)],
                33: [piece_qproj(3, 1)],
                34: [piece_qproj(3, 2)],
                35: [piece_qproj(3, 3)],
            }
            for k in range(8):
                sched[36 + k] = [piece_wo(0, k // 2, k % 2)]
                sched[44 + k] = [piece_wo(1, k // 2, k % 2)]
                sched[52 + k] = [piece_wo(2, k // 2, k % 2)]
            for k in range(4):
                sched[60 + k] = [piece_wo_ab(3, k, 0), piece_wo_ab(3, k, 1)]

            turns = []
            for c in range(4):
                for g in range(2):
                    for hp in range(NHP):
                        turns.append((g, hp, c))
            for c in range(4):
                for hp in range(NHP):
                    turns.append((2, hp, c))
            # qc3 runs hp-outer so heads finish (and normalize) early enough
            # for the first wo half to overlap the last head's attention.
            for hp in range(NHP):
                for c in range(4):
                    turns.append((3, hp, c))
            assert len(turns) == 64

            for ti, (qc, hp, c) in enumerate(turns):
                turn(qc, hp, c, sched.pop(ti, []))
            assert not sched, sched

            while pend:
                emit_pv(pend.pop(0))
            for k in range(8):
                piece_wo_cd(3, k // 2, k % 2)()

    nc.compile()
    return nc


def kernel(query, key, value, w_q, w_k, w_v, w_o):
    import ml_dtypes
    from concourse.bass_utils import run_bass_kernel_spmd

    if "nc" not in _cached:
        _cached["nc"] = _build()
    nc = _cached["nc"]

    bf = ml_dtypes.bfloat16
    query = np.asarray(query, dtype=np.float32)
    key = np.asarray(key, dtype=np.float32)
    value = np.asarray(value, dtype=np.float32)
    w_q = np.asarray(w_q, dtype=np.float32)
    w_k = np.asarray(w_k, dtype=np.float32)
    w_v = np.asarray(w_v, dtype=np.float32)
    w_o = np.asarray(w_o, dtype=np.float32)

    def c(a):
        return np.ascontiguousarray(a).astype(bf)

    in_maps = []
    for core in range(N_CORES):
        b, g = core // G, core % G
        rows = slice(DV * g, DV * (g + 1))
        in_maps.append({
            "xqT": c(query[b].T),
            "xkT": c(key[b].T),
            "xvT": c(value[b].T),
            "wqT": c(w_q[rows, :].T),
            "wkT": c(w_k[rows, :].T),
            "wvT": c(w_v[rows, :].T),
            "woT": c(w_o[:, rows].T),
        })

    res = run_bass_kernel_spmd(nc, in_maps, list(range(N_CORES)))
    full = np.empty((B, S, D), np.float32)
    for b in range(B):
        full[b] = (res.results[G * b]["out"].astype(np.float32)
                   + res.results[G * b + 1]["out"].astype(np.float32))
    return full
